# revision 31
# baseline (speedup 1.0000x reference)
"""Block-diagonal grouped GEMM (BlockDense) for Trainium2, 8 NeuronCores.

Problem: x:(8192, 16384) f32, W:(1024, 16, 16) f32
         out[b, g*16+h] = relu(sum_w x[b, g*16+w] * W[g, w, h])

Strategy:
  - Data-parallel shard of the batch dim across 8 cores (1024 rows each).
  - Host stages x/W/out in fp16 (halves HBM traffic; rel err ~4e-4 vs
    the 2e-2 gate) and relayouts x feature-major so features sit on SBUF
    partitions; 8 groups are packed into one 128x128 block-diagonal
    weight supergroup so the full PE array is used.
  - Per core: per supergroup the block-diag weight tile is the PE
    stationary operand and 512 batch columns stream per matmul (N=512,
    1 cycle/col fp16) -> 256 matmuls/core. Relu+fp16-cast PSUM->SBUF on
    alternating Scalar/Vector engines; output stored feature-major with
    8KB runs, host de-transposes.
"""

import sys

import numpy as np

import concourse.bass as bass
import concourse.mybir as mybir
import concourse.tile as tile
from concourse import bacc, bass_utils
from concourse.tile_rust import add_dep_helper


def _ensure_axon_hooks_shim():
    """The bare agent image lacks antenv.axon_hooks; bass_utils imports it
    when trace=True under axon. Provide a working shim (ctypes NTFF hook if
    the axon .so supports it, else None -> tracing is skipped gracefully)."""
    try:
        import antenv.axon_hooks  # noqa: F401
        return
    except ImportError:
        pass
    import types

    hook = None
    try:
        from trn_agent_boot.trn_boot import _ntff_profile_via_ctypes

        hook = _ntff_profile_via_ctypes("/opt/axon/libaxon_pjrt.so")
    except Exception:
        hook = None
    mod = types.ModuleType("antenv.axon_hooks")
    mod.get_axon_ntff_profile_hook = lambda: hook
    mod.set_axon_ntff_profile_hook = lambda h: None
    try:
        import antenv

        antenv.axon_hooks = mod
    except ImportError:
        pass
    sys.modules["antenv.axon_hooks"] = mod


_ensure_axon_hooks_shim()

# Problem constants (hardcoded per contract; kernel.py must be self-contained)
G, W_SZ, H = 1024, 16, 16
B = 8192
F = G * W_SZ  # 16384 input features = output features (H == W_SZ)
N_CORES = 8
B_LOC = B // N_CORES  # 1024 batch rows per core

P = 128          # partitions
GROUPS_PER_SG = 128 // W_SZ   # 8 groups per 128x128 supergroup
N_SG = G // GROUPS_PER_SG     # 128 supergroups
SG_PER_BLK = 8                # supergroups per column block
N_BLK = N_SG // SG_PER_BLK    # 16 column blocks of 1024 columns
BLK_COLS = SG_PER_BLK * P     # 1024
BT = B_LOC // P               # 8 batch tiles per core

_cached = {}

# experiment knobs (bench only; defaults are the shipping config)
CONFIG = {
    "out_engine": "scalar",  # sync | scalar  (which HWDGE ring issues stores)
    "split_x": 1,            # pieces per x-block DMA
    "x_bufs": 6,
    "o_bufs": 6,
    "relu_mix": "alt",       # alt | act | dve
    "io_dtype": "f16",       # f16 | f32  (HBM staging dtype for x, W, out)
    "mm_dtype": "io",        # io | f32 | f32r  (PE matmul input dtype)
    "layout": "sg512",       # sg512: weights-stationary N=512 | bt128: legacy
    "sg_per_store": 4,       # sg512 only: supergroups per output store
    "w_expand": "host",      # sg512 only: host (4MB pre-expanded) | chip
    "w_pieces": 8,           # host w_expand: DMA pieces for the weight tile
    "psum_wide": 1,          # sg512 only: 1 = one [128,1024] relu per sg
    "pair_blks": 1,          # bt128 only: pair column blocks
    "serial_x": 0,           # 1: chain x loads so they complete in order
}


def _build_program():
    """Build the (single-core SPMD) bass program once per process."""
    key = tuple(sorted(CONFIG.items()))
    if key in _cached:
        return _cached[key]

    f32 = mybir.dt.float32
    iodt = mybir.dt.float16 if CONFIG["io_dtype"] == "f16" else f32
    if CONFIG["mm_dtype"] == "io":
        mdt = iodt
    elif CONFIG["mm_dtype"] == "f32r":
        mdt = mybir.dt.float32r
    else:
        mdt = f32
    nc = bacc.Bacc("TRN2", debug=False, target_bir_lowering=False)

    xt_d = nc.dram_tensor("xt", (N_BLK, P, SG_PER_BLK * B_LOC), iodt,
                          kind="ExternalInput")
    # compact weights (0.5 MB f16):
    #   sg512:  [jj, w, h, sg]  -> wt columns enumerate (m=16jj+h, sg) so a
    #           per-sg stationary slice is a single-free-dim AP (BIR rule)
    #   bt128:  [jj, w, sg, h]  -> wt columns enumerate (jj, sg, h)
    host_w = CONFIG["layout"] == "sg512" and CONFIG["w_expand"] == "host"
    if host_w:
        # block-diagonal weights pre-expanded on host, sg-major columns:
        # wexp[p, sg*128 + m], nonzero iff p//16 == m//16
        wc_d = nc.dram_tensor("wc", (P, N_SG * P), iodt,
                              kind="ExternalInput")
    elif CONFIG["layout"] == "sg512":
        wc_d = nc.dram_tensor("wc", (GROUPS_PER_SG, W_SZ, H, N_SG), iodt,
                              kind="ExternalInput")
    else:
        wc_d = nc.dram_tensor("wc", (GROUPS_PER_SG, W_SZ, N_SG, H), iodt,
                              kind="ExternalInput")
    if CONFIG["layout"] == "sg512":
        # feature-major output mirroring the SBUF store tiles exactly:
        # out[si, p, sgl*B_LOC + b] = y[b, f] with
        #   f = blk*1024 + j*128 + p,  j = (si%spb)*sps + sgl,  blk = si//spb
        sps = CONFIG["sg_per_store"]
        out_d = nc.dram_tensor("out", (N_SG // sps, P, sps * B_LOC), iodt,
                               kind="ExternalOutput")
    else:
        out_d = nc.dram_tensor("out", (B_LOC, F), iodt, kind="ExternalOutput")

    xt_ap = xt_d.ap()
    wc_ap = wc_d.ap()
    out_ap = out_d.ap()

    relu = mybir.ActivationFunctionType.Relu

    out_dma = nc.scalar if CONFIG["out_engine"] == "scalar" else nc.sync

    with tile.TileContext(nc) as tc:
        with (
            tc.tile_pool(name="wpool", bufs=1) as wpool,
            tc.tile_pool(name="xpool", bufs=CONFIG["x_bufs"]) as xpool,
            tc.tile_pool(name="opool", bufs=CONFIG["o_bufs"]) as opool,
            tc.tile_pool(name="pspool",
                         bufs=(4 if CONFIG["layout"] == "sg512"
                               and CONFIG["psum_wide"] else 8),
                         space=bass.MemorySpace.PSUM) as pspool,
        ):
            # Build the resident block-diagonal weight tile once. Layout
            # groups each jj's data contiguously so the expansion DMA writes
            # one 8KB run per partition:
            #   wt_all[i, jj*2048 + sg*16 + h] = W[8*sg+jj, w, h]  (i = 16jj+w)
            # The matmul rhs for supergroup sg reads it back with a strided
            # 3-D AP whose (jj, h) enumeration equals output column o=16jj+h.
            wt_all = wpool.tile([P, N_SG * P], iodt)
            wt_rhs = None
            if host_w:
                # piecewise load of the pre-expanded tile; each matmul only
                # depends on the piece covering its sg columns.
                npc = CONFIG["w_pieces"]
                pc = (N_SG * P) // npc
                for i in range(npc):
                    out_dma.dma_start(wt_all[:, i * pc:(i + 1) * pc],
                                      wc_ap[:, i * pc:(i + 1) * pc])
            else:
                blk2 = N_SG * H  # 2048
                # Per-jj memset then per-jj weight DMA: each DMA only waits
                # on its own column range, so the expansion pipelines
                # instead of stalling on one full-tile memset barrier.
                # Keep scalar free: it issues the wc DMA triggers (and later
                # stores), so memsets run on vector/gpsimd only.
                if CONFIG["layout"] == "sg512":
                    ms_engines = [nc.vector, nc.gpsimd]
                else:
                    ms_engines = [nc.vector, nc.scalar, nc.gpsimd]
                for jj in range(GROUPS_PER_SG):
                    eng = ms_engines[jj % len(ms_engines)]
                    seg = wt_all[:, jj * blk2:(jj + 1) * blk2]
                    if eng is nc.scalar:
                        eng.memzero(seg)
                    else:
                        eng.memset(seg, 0.0)
                    out_dma.dma_start(
                        wt_all[16 * jj:16 * jj + 16,
                               jj * blk2:(jj + 1) * blk2],
                        wc_ap[jj],
                    )
                if CONFIG["layout"] == "sg512":
                    # columns are (m, sg) with m = 16*jj + h
                    wt_rhs = wt_all[:].rearrange("p (m sg) -> p m sg", m=P)
                else:
                    wt_rhs = wt_all[:].rearrange("p (jj sg h) -> p jj sg h",
                                                 jj=GROUPS_PER_SG, h=H)

            def compute_halves(xt_t, blk, bt, ot, o_off):
                for half in range(2):
                    ps = pspool.tile([P, 512], f32)
                    for q in range(4):
                        j = half * 4 + q
                        sg = blk * SG_PER_BLK + j
                        lhsT = xt_t[:, j * B_LOC + bt * P:
                                    j * B_LOC + bt * P + P]
                        rhs = wt_rhs[:, :, sg, :]
                        if mdt is not iodt:
                            lhsT = lhsT.bitcast(mdt)
                            rhs = rhs.bitcast(mdt)
                        nc.tensor.matmul(ps[:, q * P:(q + 1) * P],
                                         lhsT, rhs,
                                         start=True, stop=True)
                    dst = ot[:, o_off + half * 512:o_off + (half + 1) * 512]
                    mix = CONFIG["relu_mix"]
                    use_act = (mix == "act" or
                               (mix == "alt" and (bt * 2 + half) % 2 == 0))
                    if use_act:
                        nc.scalar.activation(dst, ps[:], relu)
                    else:
                        nc.vector.tensor_scalar_max(dst, ps[:], 0.0)

            prev_load = [None]

            def load_x(blk):
                xt_t = xpool.tile([P, SG_PER_BLK * B_LOC], iodt)
                # finer pieces for the first pair so compute starts sooner
                nsp = 2 if blk < 2 else CONFIG["split_x"]
                piece = (SG_PER_BLK * B_LOC) // nsp
                for sp in range(nsp):
                    di = nc.sync.dma_start(
                        xt_t[:, sp * piece:(sp + 1) * piece],
                        xt_ap[blk, :, sp * piece:(sp + 1) * piece],
                    )
                    if CONFIG["serial_x"]:
                        if prev_load[0] is not None:
                            add_dep_helper(di.ins, prev_load[0],
                                           reason="serialize x loads")
                        prev_load[0] = di.ins
                return xt_t

            if CONFIG["layout"] == "sg512":
                # Weights-stationary grouped GEMM: per supergroup sg, the
                # 128x128 block-diagonal weight tile is the stationary
                # operand and 512 batch columns stream per matmul (N=512,
                # 1 cycle/col at fp16). 256 matmuls total instead of 1024.
                sps = CONFIG["sg_per_store"]
                n_ot = SG_PER_BLK // sps
                relu_i = [0]
                for blk in range(N_BLK):
                    xt_t = load_x(blk)
                    for oi in range(n_ot):
                        ot = opool.tile([P, sps * B_LOC], iodt)
                        for sgl in range(sps):
                            j = oi * sps + sgl
                            sg = blk * SG_PER_BLK + j
                            if host_w:
                                lhsT = wt_all[:, sg * P:(sg + 1) * P]
                            else:
                                lhsT = wt_rhs[:, :, sg]
                            if mdt is not iodt:
                                lhsT = lhsT.bitcast(mdt)
                            wide = CONFIG["psum_wide"]
                            ps = pspool.tile([P, 1024 if wide else 512], f32)
                            for half in range(2):
                                if not wide and half == 1:
                                    ps = pspool.tile([P, 512], f32)
                                pdst = (ps[:, half * 512:(half + 1) * 512]
                                        if wide else ps[:])
                                rhs = xt_t[:, j * B_LOC + half * 512:
                                           j * B_LOC + half * 512 + 512]
                                if mdt is not iodt:
                                    rhs = rhs.bitcast(mdt)
                                nc.tensor.matmul(pdst, lhsT, rhs,
                                                 start=True, stop=True)
                                if wide and half == 0:
                                    continue
                                if wide:
                                    dst = ot[:, sgl * B_LOC:
                                             (sgl + 1) * B_LOC]
                                    src = ps[:]
                                else:
                                    dst = ot[:, sgl * B_LOC + half * 512:
                                             sgl * B_LOC + (half + 1) * 512]
                                    src = ps[:]
                                mix = CONFIG["relu_mix"]
                                use_act = (mix == "act" or
                                           (mix == "alt" and
                                            relu_i[0] % 2 == 0))
                                relu_i[0] += 1
                                if use_act:
                                    nc.scalar.activation(dst, src, relu)
                                else:
                                    nc.vector.tensor_scalar_max(dst, src,
                                                                0.0)
                        out_dma.dma_start(out_ap[blk * n_ot + oi], ot[:])
            elif CONFIG["pair_blks"]:
                for pair in range(N_BLK // 2):
                    xts = [load_x(pair * 2), load_x(pair * 2 + 1)]
                    for bt in range(BT):
                        ot = opool.tile([P, 2 * BLK_COLS], iodt)
                        for u in range(2):
                            compute_halves(xts[u], pair * 2 + u, bt, ot,
                                           u * BLK_COLS)
                        out_dma.dma_start(
                            out_ap[bt * P:(bt + 1) * P,
                                   pair * 2 * BLK_COLS:
                                   (pair + 1) * 2 * BLK_COLS],
                            ot[:],
                        )
            else:
                for blk in range(N_BLK):
                    xt_t = load_x(blk)
                    for bt in range(BT):
                        ot = opool.tile([P, BLK_COLS], iodt)
                        compute_halves(xt_t, blk, bt, ot, 0)
                        out_dma.dma_start(
                            out_ap[bt * P:(bt + 1) * P,
                                   blk * BLK_COLS:(blk + 1) * BLK_COLS],
                            ot[:],
                        )

    nc.compile()
    _cached[key] = nc
    return nc


def _io_np_dtype():
    return np.float16 if CONFIG["io_dtype"] == "f16" else np.float32


def _prep_w(W: np.ndarray) -> np.ndarray:
    """Compact weights reordered for the on-chip block-diag expansion."""
    Wr = np.ascontiguousarray(W, dtype=np.float32).reshape(
        N_SG, GROUPS_PER_SG, W_SZ, H)
    if CONFIG["layout"] == "sg512" and CONFIG["w_expand"] == "host":
        # wexp[16jj+w, sg*128 + 16jj+h] = W[8sg+jj, w, h], zero elsewhere
        wexp = np.zeros((GROUPS_PER_SG, W_SZ, N_SG, GROUPS_PER_SG, H),
                        dtype=np.float32)
        jj = np.arange(GROUPS_PER_SG)
        wexp[jj, :, :, jj, :] = Wr.transpose(1, 2, 0, 3)
        return np.ascontiguousarray(
            wexp.reshape(P, N_SG * P).astype(_io_np_dtype()))
    if CONFIG["layout"] == "sg512":
        perm = (1, 2, 3, 0)   # [jj, w, h, sg]
    else:
        perm = (1, 2, 0, 3)   # [jj, w, sg, h]
    return np.ascontiguousarray(
        Wr.transpose(*perm).astype(_io_np_dtype()))


def _prep_x_shard(xs: np.ndarray) -> np.ndarray:
    """Relayout one (1024, 16384) shard to (16, 128, 8*1024).

    xt[blk, p, j*1024 + b] = xs[b, blk*1024 + j*128 + p]
    """
    x4 = xs.reshape(B_LOC, N_BLK, SG_PER_BLK, P)          # b, blk, j, p
    xt = np.ascontiguousarray(
        x4.transpose(1, 3, 2, 0).astype(_io_np_dtype()))   # blk, p, j, b
    return xt.reshape(N_BLK, P, SG_PER_BLK * B_LOC)


# Debug/benchmark knobs (used by test.py only; harness leaves defaults)
TRACE = False
TRACE_CORES = None  # e.g. [0] or list(range(8))
LAST_RESULTS = None


def kernel(x: np.ndarray, W: np.ndarray) -> np.ndarray:
    global LAST_RESULTS
    assert x.shape == (B, F) and W.shape == (G, W_SZ, H)
    x = np.ascontiguousarray(x, dtype=np.float32)

    wc = _prep_w(W)
    in_maps = []
    for s in range(N_CORES):
        xs = x[s * B_LOC:(s + 1) * B_LOC]
        in_maps.append({"xt": _prep_x_shard(xs), "wc": wc})

    nc = _build_program()
    kwargs = {}
    if TRACE:
        kwargs = {"trace": True, "trace_cores": TRACE_CORES}
    res = bass_utils.run_bass_kernel_spmd(nc, in_maps,
                                          core_ids=list(range(N_CORES)),
                                          **kwargs)
    LAST_RESULTS = res
    if CONFIG["layout"] == "sg512":
        sps = CONFIG["sg_per_store"]
        n_ot = SG_PER_BLK // sps
        arr = np.stack([r["out"] for r in res.results])
        # (core, si, p, sgl*B_LOC+b) -> (core, b, blk, oi, sgl, p)
        arr = arr.reshape(N_CORES, N_BLK, n_ot, P, sps, B_LOC)
        out = np.ascontiguousarray(
            arr.transpose(0, 5, 1, 2, 4, 3), dtype=np.float32)
        out = out.reshape(B, F)
    else:
        out = np.concatenate([r["out"] for r in res.results], axis=0)
        if out.dtype != np.float32:
            out = out.astype(np.float32)
    return out

